# revision 36
# baseline (speedup 1.0000x reference)
"""BitLinear kernel for Trainium2 (8 NeuronCores, tensor-parallel).

Computes: out = x @ (sign(w) * mean(|w|, axis=1, keepdims=True)).T
  x      : [4, 2048, 4096] f32
  weight : [4096, 4096] f32
  out    : [4, 2048, 4096] f32

Strategy: shard weight rows (out features) 8-way; each core computes a
[512, 8192] feature-major output shard. Measured ~335.5-336.7us HW
exec, rel-err 1.983e-2 (gate 2e-2).

Cost model (all measured on HW): EVERY matmul instruction — fp16
1-ktile or fp8e4 DoubleRow 2-ktile — issues at the same ~216ns for 512
moving elements (518 cycles; the old +19ns-per-DR estimate was wrong),
so exec time = instruction count x 216ns + head + tail:
  exec ≈ preamble (~7us, fixed HW handshakes: CSR write + 2.5us event
  wait, 2 all-engine barriers, uncached register loads, queue init)
       + first-chunk DMA wait (~3us: cold-HBM per-queue rate starts
         ~45GB/s and ramps to ~165+GB/s by ~20us)
       + 1472 x 216ns stream (64 output tiles x (14 fp16 + 9 DR))
       + ~5us of per-pair PE hiccups locked to the big incoming x DMA
         bursts (~620ns when the next pair's 3.7MB xb DMA starts,
         ~430ns when its xf DMA starts — SBUF write contention;
         instruction reordering does not move them)
       + ~1.5us residual startup stalls
       + 5.7us fixed tail (the profiler's "useful time" window closes
         a constant ~5.65-5.77us after the LAST MATMUL, regardless of
         eviction/store/teardown structure — only the matmul stream
         end matters).

Precision split: 14 fp16 k-tiles + 18 e4m3 k-tiles (9 DoubleRow
pairs). sign(w) is exact in fp16/fp8e4; all error comes from e4m3 x
(per-element RMS ~2.65e-2) -> rel-err 2.65e-2*sqrt(18/32) = 1.983e-2.
n_f=20 would give 2.10e-2 (fails); e3m4 (float8e3, 4 mantissa bits)
with DoubleRow would fix that but the backend rejects it (DR is
e4m3/e5m2-only, matching the CoreSim cost model). DoubleColumn/
DoublePixel are unimplemented in bass. x/signs/out use fp16 (not bf16)
to zero out the non-fp8 error terms for free.

Startup (the only schedule-sensitive part): the PE clock (HAM) needs
~3.4us of CONTINUOUS busy to reach full speed and an idle gap resets
the credit, so ~3.4us of back-to-back warmup matmuls run from
preamble-end straight into the first real matmul. Pair 0 is consumed
j-outer across BOTH 512-token blocks (8 PSUM banks, halves the
required supply rate) from single-k-tile 256KB chunks (2KB contiguous
per partition; 1KB-packet strided chunks run ~3x slower); sync carries
x, scalar carries sct (FIRST — at the chain tail this 2KB DMA lands
~25us late and stalls the first eviction) + signs + one x chunk.
Loading the two queues asymmetrically matters: they share the ramping
HBM pipe, and an overloaded scalar queue starves pair 0's late DR
phases (costs ~3-5us plus a HAM downclock). gpsimd DMAs are software-
executed (~7-11us completion) — never on the critical path.

Steady state: per 1024-token pair, x rides ONE big DMA per dtype;
per-block j-then-DR accumulation into 4 banks, eviction (scale via
ACTIVATE/TENSOR_SCALAR alternating scalar/vector engines) overlaps the
other block, one [128,4,1024] store per pair. Semaphore teardown
(~250 clears) and preamble are fixed costs — DMA-count reduction does
NOT shrink them (the full semaphore file is always reset).
"""

import os
from contextlib import ExitStack

import numpy as np
import ml_dtypes

import concourse.bass as bass
import concourse.mybir as mybir
import concourse.tile as tile
from concourse import bacc, bass_utils

P = 128                 # SBUF partitions / PE array dim
D_IN = 4096             # contraction dim (in features)
D_OUT = 4096            # out features
M_TOT = 8192            # tokens (4*2048)
N_CORES = 8
N_SHARD = D_OUT // N_CORES      # 512 out features per core
K_TILES = D_IN // P             # 32
NB = 14                         # fp16 k-tiles (0..NB-1)
NFP = (K_TILES - NB) // 2       # 9 fp8 DoubleRow k-tile pairs
M_BLK = 512                     # moving free dim per matmul
M_BLKS = M_TOT // M_BLK         # 16
M_PAIRS = M_BLKS // 2           # 8 (x is loaded in block pairs)
N_TILES = N_SHARD // P          # 4
PAIR_W = 2 * M_BLK              # 1024

_CACHE = {}
LAST_RESULTS = None  # BassKernelResults of the most recent run (for test harness)


def _install_ntff_hook():
    """Register the ctypes NTFF profiling hook under antenv.axon_hooks so
    run_bass_kernel_spmd(trace=True) can capture device profiles under axon.
    No-op if already present or the .so lacks the symbols."""
    import contextlib
    import ctypes
    import sys
    import types

    try:
        from antenv.axon_hooks import get_axon_ntff_profile_hook  # noqa: F401

        return True
    except ImportError:
        pass

    so_path = "/opt/axon/libaxon_pjrt.so"
    if not os.path.exists(so_path):
        return False
    lib = ctypes.CDLL(so_path)
    if not hasattr(lib, "axon_start_nrt_profile"):
        return False
    lib.axon_start_nrt_profile.argtypes = [
        ctypes.POINTER(ctypes.c_int64),
        ctypes.c_size_t,
    ]
    lib.axon_start_nrt_profile.restype = ctypes.c_int64
    lib.axon_stop_nrt_profile.argtypes = [ctypes.c_char_p]
    lib.axon_stop_nrt_profile.restype = ctypes.c_int64

    @contextlib.contextmanager
    def _hook(output_dir, device_ids):
        import jax

        jax.devices()
        if device_ids:
            ids = (ctypes.c_int64 * len(device_ids))(*device_ids)
            rc = lib.axon_start_nrt_profile(ids, len(device_ids))
        else:
            rc = lib.axon_start_nrt_profile(None, 0)
        if rc != 0:
            raise RuntimeError(f"axon_start_nrt_profile rc={rc}")
        try:
            yield
        finally:
            n = lib.axon_stop_nrt_profile(str(output_dir).encode())
            print(f"ntff profile: {n} file(s) written to {output_dir}")

    mod = types.ModuleType("antenv.axon_hooks")
    _state = {"hook": _hook}
    mod.set_axon_ntff_profile_hook = lambda h: _state.__setitem__("hook", h)
    mod.get_axon_ntff_profile_hook = lambda: _state["hook"]
    sys.modules["antenv.axon_hooks"] = mod
    import antenv

    antenv.axon_hooks = mod

    # artifact upload reaches for a cloud bucket that isn't available here
    bass_utils.upload_artifacts = lambda tmpdir: f"local:{tmpdir}"
    return True


def _build_nc():
    nc = bacc.Bacc(
        "TRN2", target_bir_lowering=False, debug=False, num_devices=N_CORES,
        enable_partition_id=False,
    )
    xH = nc.dram_tensor(
        "xH", [M_PAIRS, P, NB, PAIR_W], mybir.dt.float16, kind="ExternalInput"
    )
    xF = nc.dram_tensor(
        "xF", [M_PAIRS, P, NFP, 2, PAIR_W], mybir.dt.float8e4,
        kind="ExternalInput",
    )
    sgB = nc.dram_tensor(
        "sgB", [P, NB * N_SHARD], mybir.dt.float16, kind="ExternalInput"
    )
    sgF = nc.dram_tensor(
        "sgF", [P, NFP * 2 * N_SHARD], mybir.dt.float8e4, kind="ExternalInput"
    )
    sc = nc.dram_tensor("sc", [P, N_TILES], mybir.dt.float32, kind="ExternalInput")
    outT = nc.dram_tensor(
        "outT", [P, N_TILES, M_TOT], mybir.dt.float16, kind="ExternalOutput"
    )

    with tile.TileContext(nc) as tc, ExitStack() as ctx:
        sb = ctx.enter_context(tc.tile_pool(name="sb", bufs=1))
        pp = ctx.enter_context(tc.tile_pool(name="psum", bufs=1, space="PSUM"))

        # Every tile allocated exactly once; reuse is explicit by parity.
        sgB_t = sb.tile([P, NB, N_SHARD], mybir.dt.float16)
        sgF_t = sb.tile([P, NFP, 2, N_SHARD], mybir.dt.float8e4)
        sct = sb.tile([P, N_TILES], mybir.dt.float32)
        warm = sb.tile([P, P + M_BLK], mybir.dt.float16)
        # x tiles are TRIPLE-buffered: with 2 buffers pair q's load can only
        # start after pair q-2 is fully read and completes just-in-time,
        # costing ~0.4us of semaphore latency at every other pair boundary.
        xbs = [
            sb.tile([P, NB, PAIR_W], mybir.dt.float16, name=f"xb{i}")
            for i in range(3)
        ]
        xfs = [
            sb.tile([P, NFP, 2, PAIR_W], mybir.dt.float8e4, name=f"xf{i}")
            for i in range(3)
        ]
        ops = [
            sb.tile([P, N_TILES, PAIR_W], mybir.dt.float16, name=f"op{i}")
            for i in range(2)
        ]
        # 8 PSUM banks as two 4-bank sets; block g (= 2q+b) uses set g%2.
        psums = [
            [
                pp.tile([P, M_BLK], mybir.dt.float32, name=f"ps{i}_{ni}")
                for ni in range(N_TILES)
            ]
            for i in range(2)
        ]

        # Per-queue DMA emission-order chains.
        prev_dma = {}

        def qload(queue, qname, dst, src):
            dma = queue.dma_start(dst, src)
            if prev_dma.get(qname) is not None:
                tile.add_dep_helper(
                    dma.ins, prev_dma[qname].ins, sync=False,
                    reason="DMA queue emission order",
                )
            prev_dma[qname] = dma
            return dma

        # ---- HAM warmup: three dummy matmuls with no DMA dependencies give
        # the PE clock-gate monitor activity credit while the first x/sign
        # chunks land; sized to finish as the operands become consumable.
        # ~3.4us of back-to-back cold-clock warmups — exactly the HAM ramp
        # window — so the real stream starts at full clock; an idle gap
        # between warmups and the first data-ready matmul resets the ramp
        # credit. The tail warmups are 256-wide so the first real matmul
        # slots in behind the in-order queue with finer granularity.
        nc.vector.memset(warm[:], 0.0)
        for wi, w in enumerate((M_BLK,) * 5 + (M_BLK // 2,) * 5):
            nc.tensor.matmul(
                psums[0][wi % 3][:, 0:w], warm[:, 0:P], warm[:, P : P + w],
                start=True, stop=True,
            )

        xb0, xf0 = xbs[0], xfs[0]
        # The DMA/HBM path itself ramps (~45GB/s cold to ~165+GB/s after
        # ~15us), so pair 0 is consumed j-outer across BOTH blocks (half the
        # supply rate of the per-block order) and its x arrives as full-j
        # 256KB chunks (2KB contiguous per partition) — except j0, split in
        # halves so the first matmul fires ~2.7us earlier. (gpsimd DMAs are
        # software-executed with multi-us latency — never on the critical
        # path.)
        # Early supply is limited by the cold-HBM per-queue DMA rate
        # (~45GB/s ramping to ~165+), and only sync + scalar have hardware
        # DMA queues; weave a third of the x chunks into the scalar queue
        # between the (half-sized) sign chunks so both queues carry ~equal
        # bytes per j-phase, everything in consumption order.
        qload(nc.sync, "y", xb0[:, 0, 0:M_BLK], xH[0, :, 0, 0:M_BLK])
        qload(nc.sync, "y", xb0[:, 0, M_BLK:PAIR_W], xH[0, :, 0, M_BLK:PAIR_W])
        for j in (1, 2, 3, 5, 6, 7):
            qload(nc.sync, "y", xb0[:, j, :], xH[0, :, j, :])
        for j0 in range(8, NB, 2):
            j1 = min(j0 + 2, NB)
            qload(nc.sync, "y", xb0[:, j0:j1, :], xH[0, :, j0:j1, :])
        for c in range(3):
            qload(nc.sync, "y", xf0[:, c, :, :], xF[0, :, c, :, :])
        for c0 in range(3, NFP, 3):
            c1 = min(c0 + 3, NFP)
            qload(nc.sync, "y", xf0[:, c0:c1, :, :], xF[0, :, c0:c1, :, :])
        # Scalar: sct first — it's 2KB but the first eviction (~52us) needs
        # it, and at the chain's tail it completes WAY too late (~55us) —
        # then signs in consumption order. Don't put x chunks here: the two
        # queues share the ramping HBM pipe and a loaded scalar queue
        # starves q0's late DR phases.
        qload(nc.scalar, "s", sct[:], sc[:, :])
        qload(nc.scalar, "s", sgB_t[:, 0, :], sgB[:, 0:N_SHARD])
        qload(nc.scalar, "s", sgB_t[:, 1, :], sgB[:, N_SHARD : 2 * N_SHARD])
        qload(nc.scalar, "s", sgB_t[:, 2:4, :], sgB[:, 2 * N_SHARD : 4 * N_SHARD])
        # one x chunk rides scalar (after the signs it must not delay) to
        # relieve the cold sync queue mid-startup
        qload(nc.scalar, "s", xb0[:, 4, :], xH[0, :, 4, :])
        qload(nc.scalar, "s", sgB_t[:, 4:7, :], sgB[:, 4 * N_SHARD : 7 * N_SHARD])
        qload(nc.scalar, "s", sgB_t[:, 7:11, :], sgB[:, 7 * N_SHARD : 11 * N_SHARD])
        qload(nc.scalar, "s", sgB_t[:, 11:NB, :], sgB[:, 11 * N_SHARD : NB * N_SHARD])
        qload(nc.scalar, "s", sgF_t[:, 0:3, :, :], sgF[:, 0 : 6 * N_SHARD])
        qload(nc.scalar, "s", sgF_t[:, 3:6, :, :], sgF[:, 6 * N_SHARD : 12 * N_SHARD])
        qload(nc.scalar, "s", sgF_t[:, 6:NFP, :, :], sgF[:, 12 * N_SHARD :])

        def issue_x_pair(q):
            # Both x DMAs stay on the sync queue: routing xf via scalar
            # parks it behind store_pair(q) (which blocks until pair q's
            # evictions), making xf borderline-late every pair — stalls
            # plus a HAM downclock cascade (~+60us!).
            xb, xf = xbs[q % 3], xfs[q % 3]
            qload(nc.sync, "y", xb[:, :, :], xH[q, :, :, :])
            qload(nc.sync, "y", xf[:, :, :, :], xF[q, :, :, :, :])
            return xb, xf

        def mm_b(pss, xb, b, ni, j, start=None, stop=False):
            nc.tensor.matmul(
                pss[ni][:],
                sgB_t[:, j, ni * P : (ni + 1) * P],
                xb[:, j, b * M_BLK : (b + 1) * M_BLK],
                start=(j == 0) if start is None else start,
                stop=stop,
            )

        def mm_f(pss, xf, b, ni, jj, start=False, stop=None):
            nc.tensor.matmul(
                pss[ni][:],
                sgF_t[:, jj, :, ni * P : (ni + 1) * P],
                xf[:, jj, :, b * M_BLK : (b + 1) * M_BLK],
                start=start,
                stop=(jj == NFP - 1) if stop is None else stop,
                perf_mode=mybir.MatmulPerfMode.DoubleRow,
            )

        def do_block(pss, xb, xf, b, dr_first, ni_list=None):
            # Adjacent blocks run their fp16/DR sections in opposite order
            # so block seams meet same-mode (measured neutral — the per-pair
            # hiccups are DMA-burst-locked, not mode-switch — but harmless);
            # start/stop accumulation flags ride whichever section runs
            # first/last.
            nis = range(N_TILES) if ni_list is None else ni_list
            if dr_first:
                for jj in range(NFP):
                    for ni in nis:
                        mm_f(pss, xf, b, ni, jj, start=(jj == 0), stop=False)
                for j in range(NB):
                    for ni in nis:
                        mm_b(pss, xb, b, ni, j, start=False, stop=(j == NB - 1))
            else:
                for j in range(NB):
                    for ni in nis:
                        mm_b(pss, xb, b, ni, j)
                for jj in range(NFP):
                    for ni in nis:
                        mm_f(pss, xf, b, ni, jj)

        def evict_block(pss, op, b):
            # Evictions alternate between the scalar and vector engines so
            # the per-block eviction chain (and the kernel tail) is half as
            # long.
            for ni in range(N_TILES):
                dst = op[:, ni, b * M_BLK : (b + 1) * M_BLK]
                if ni % 2 == 0:
                    nc.scalar.mul(dst, pss[ni][:], sct[:, ni : ni + 1])
                else:
                    nc.vector.tensor_scalar_mul(dst, pss[ni][:], sct[:, ni : ni + 1])

        def store_pair(q, op):
            qload(
                nc.scalar, "s",
                outT[:, :, q * PAIR_W : (q + 1) * PAIR_W], op[:, :, :],
            )

        # ---- Main loop
        for q in range(M_PAIRS):
            xb, xf = (xb0, xf0) if q == 0 else issue_x_pair(q)
            op = ops[q % 2]
            if q == 0:
                # j-outer across BOTH blocks (8 PSUM banks) so the PE keeps
                # pace with the ramping HBM-limited startup stream.
                for j in range(NB):
                    for b in range(2):
                        for ni in range(N_TILES):
                            mm_b(psums[b], xb, b, ni, j)
                for jj in range(NFP):
                    # natural b-order: psums[0]'s stop lands 8 instructions
                    # before q0's end, so its eviction overlaps the tail and
                    # pair 1's first matmuls start on freed banks
                    for b in range(2):
                        for ni in range(N_TILES):
                            mm_f(psums[b], xf, b, ni, jj)
                for b in range(2):
                    evict_block(psums[b], op, b)
                store_pair(q, op)
            elif q < M_PAIRS - 1:
                for b in range(2):
                    pss = psums[b]
                    do_block(pss, xb, xf, b, dr_first=(b == 1))
                    evict_block(pss, op, b)
                store_pair(q, op)
            else:
                # Final pair: block 0 stores as one chunk as soon as its
                # eviction completes; block 1 runs ni-outer so each n-tile's
                # stop matmul lands early and its eviction + store overlap
                # the remaining matmuls; the very last n-tile's eviction and
                # store are split across both engines / two DMA queues.
                pss = psums[0]
                do_block(pss, xb, xf, 0, dr_first=False)
                evict_block(pss, op, 0)
                qload(
                    nc.scalar, "s",
                    outT[:, :, q * PAIR_W : q * PAIR_W + M_BLK],
                    op[:, :, 0:M_BLK],
                )
                pss = psums[1]
                tailq = [(nc.scalar, "s"), (nc.sync, "y"), (nc.scalar, "s")]
                for ni in range(N_TILES):
                    # per-ni alternation keeps every ni seam same-mode
                    do_block(pss, xb, xf, 1, dr_first=(ni % 2 == 0), ni_list=[ni])
                    c0 = q * PAIR_W + M_BLK
                    if ni < N_TILES - 1:
                        dst = op[:, ni, M_BLK:PAIR_W]
                        if ni % 2 == 0:
                            nc.scalar.mul(dst, pss[ni][:], sct[:, ni : ni + 1])
                        else:
                            nc.vector.tensor_scalar_mul(
                                dst, pss[ni][:], sct[:, ni : ni + 1]
                            )
                        eng, en = tailq[ni]
                        qload(
                            eng, en,
                            outT[:, ni, c0 : c0 + M_BLK],
                            op[:, ni, M_BLK:PAIR_W],
                        )
                    else:
                        # split the last eviction + store across engines
                        H = M_BLK // 2
                        nc.scalar.mul(
                            op[:, ni, M_BLK : M_BLK + H],
                            pss[ni][:, 0:H], sct[:, ni : ni + 1],
                        )
                        nc.vector.tensor_scalar_mul(
                            op[:, ni, M_BLK + H : PAIR_W],
                            pss[ni][:, H:M_BLK], sct[:, ni : ni + 1],
                        )
                        qload(
                            nc.scalar, "s",
                            outT[:, ni, c0 : c0 + H],
                            op[:, ni, M_BLK : M_BLK + H],
                        )
                        qload(
                            nc.sync, "y",
                            outT[:, ni, c0 + H : c0 + M_BLK],
                            op[:, ni, M_BLK + H : PAIR_W],
                        )

    nc.compile()
    return nc


def kernel(x, weight):
    global LAST_RESULTS
    nc = _CACHE.get("nc")
    if nc is None:
        nc = _CACHE["nc"] = _build_nc()

    x = np.asarray(x)
    weight = np.asarray(weight)
    orig_shape = x.shape

    KB = NB * P  # contraction cols in fp16

    # Host-side layout: x.T pre-tiled, partition-major so each pair is one
    # contiguous [128, *] DMA; fp16 for k-tiles 0..NB-1, e4m3 for the
    # DoubleRow k-tile pairs.
    xT = x.reshape(M_TOT, D_IN).T  # [D_IN, M_TOT] view
    # [q, p, j*1024 + c] = xT[j*128+p, q*1024+c]
    xH = np.ascontiguousarray(
        xT[:KB].reshape(NB, P, M_PAIRS, PAIR_W)
        .transpose(2, 1, 0, 3)
        .reshape(M_PAIRS, P, NB * PAIR_W)
        .astype(np.float16)
    )
    # [q, p, jj*2048 + t*1024 + c] = xT[KB + (2jj+t)*128 + p, q*1024+c]
    xF = np.ascontiguousarray(
        xT[KB:].reshape(NFP, 2, P, M_PAIRS, PAIR_W)
        .transpose(3, 2, 0, 1, 4)
        .reshape(M_PAIRS, P, NFP * 2 * PAIR_W)
        .astype(ml_dtypes.float8_e4m3fn)
    )

    SgT = np.sign(weight.T)  # [D_IN, D_OUT] f32, sign exact
    s_full = np.abs(weight.astype(np.float64)).mean(axis=1).astype(np.float32)

    in_maps = []
    for c in range(N_CORES):
        n0 = c * N_SHARD
        shard = SgT[:, n0 : n0 + N_SHARD]  # [D_IN, 512]
        # sgB[p, j*512+n] = sign(wT[j*128+p, n0+n])
        sgB = np.ascontiguousarray(
            shard[:KB].reshape(NB, P, N_SHARD)
            .transpose(1, 0, 2)
            .reshape(P, NB * N_SHARD)
            .astype(np.float16)
        )
        # sgF[p, jj*1024 + t*512 + n] = sign(wT[(NB+2jj+t)*128+p, n0+n])
        sgF = np.ascontiguousarray(
            shard[KB:].reshape(NFP, 2, P, N_SHARD)
            .transpose(2, 0, 1, 3)
            .reshape(P, NFP * 2 * N_SHARD)
            .astype(ml_dtypes.float8_e4m3fn)
        )
        scl = np.ascontiguousarray(
            s_full[n0 : n0 + N_SHARD].reshape(N_TILES, P).T
        )  # [128, 4] f32
        in_maps.append({"xH": xH, "xF": xF, "sgB": sgB, "sgF": sgF, "sc": scl})

    trace = bool(int(os.environ.get("BITLIN_TRACE", "0")))
    if trace:
        trace = _install_ntff_hook()
        base = os.environ.get("BITLIN_TRACE_DIR") or None
        if base:
            import tempfile

            os.makedirs(base, exist_ok=True)
            tmpdir = tempfile.mkdtemp(dir=base)
        else:
            tmpdir = None
    else:
        tmpdir = None
    res = bass_utils.run_bass_kernel_spmd(
        nc, in_maps, core_ids=list(range(N_CORES)), trace=trace, tmpdir=tmpdir
    )
    LAST_RESULTS = res

    # outT[c] is [128, 4, 8192] fp16 with feature index = ni*128 + p.
    outT_full = np.concatenate(
        [
            np.asarray(res.results[c]["outT"]).transpose(1, 0, 2).reshape(
                N_SHARD, M_TOT
            )
            for c in range(N_CORES)
        ],
        axis=0,
    )  # [D_OUT, M_TOT] fp16
    out = (
        np.ascontiguousarray(outT_full.T).astype(np.float32).reshape(orig_shape)
    )
    return out


# revision 37
# speedup vs baseline: 1.0041x; 1.0041x over previous
"""BitLinear kernel for Trainium2 (8 NeuronCores, tensor-parallel).

Computes: out = x @ (sign(w) * mean(|w|, axis=1, keepdims=True)).T
  x      : [4, 2048, 4096] f32
  weight : [4096, 4096] f32
  out    : [4, 2048, 4096] f32

Strategy: shard weight rows (out features) 8-way; each core computes a
[512, 8192] feature-major output shard. Measured ~335.5-336.7us HW
exec, rel-err 1.983e-2 (gate 2e-2).

Cost model (all measured on HW): EVERY matmul instruction — fp16
1-ktile or fp8e4 DoubleRow 2-ktile — issues at the same ~216ns for 512
moving elements (518 cycles; the old +19ns-per-DR estimate was wrong),
so exec time = instruction count x 216ns + head + tail:
  exec ≈ preamble (~7us, fixed HW handshakes: CSR write + 2.5us event
  wait, 2 all-engine barriers, uncached register loads, queue init)
       + first-chunk DMA wait (~3us: cold-HBM per-queue rate starts
         ~45GB/s and ramps to ~165+GB/s by ~20us)
       + 1472 x 216ns stream (64 output tiles x (14 fp16 + 9 DR))
       + ~3.5us of per-pair PE hiccups locked to the big incoming x DMA
         bursts (~620ns at each pair's 3.7MB xb DMA start — SBUF write
         contention; instruction reordering does not move them)
       + ~1us cold-clock ramp residue at stream start.
  The whole chip also lotteries into a sustained-load P0 downclock
  (2.4 -> 2.0GHz, exactly +20% on everything) under long benchmarking
  sessions — unrelated to kernel structure.
       + 5.7us fixed tail (the profiler's "useful time" window closes
         a constant ~5.65-5.77us after the LAST MATMUL, regardless of
         eviction/store/teardown structure — only the matmul stream
         end matters).

Precision split: 14 fp16 k-tiles + 18 e4m3 k-tiles (9 DoubleRow
pairs). sign(w) is exact in fp16/fp8e4; all error comes from e4m3 x
(per-element RMS ~2.65e-2) -> rel-err 2.65e-2*sqrt(18/32) = 1.983e-2.
n_f=20 would give 2.10e-2 (fails); e3m4 (float8e3, 4 mantissa bits)
with DoubleRow would fix that but the backend rejects it (DR is
e4m3/e5m2-only, matching the CoreSim cost model). DoubleColumn/
DoublePixel are unimplemented in bass. x/signs/out use fp16 (not bf16)
to zero out the non-fp8 error terms for free.

Startup (the only schedule-sensitive part): the PE clock (HAM) needs
~3.4us of CONTINUOUS busy to reach full speed and an idle gap resets
the credit, so ~3.4us of back-to-back warmup matmuls run from
preamble-end straight into the first real matmul. Pair 0 is consumed
j-outer across BOTH 512-token blocks (8 PSUM banks, halves the
required supply rate) from single-k-tile 256KB chunks (2KB contiguous
per partition; 1KB-packet strided chunks run ~3x slower); sync carries
x, scalar carries sct (FIRST — at the chain tail this 2KB DMA lands
~25us late and stalls the first eviction) + signs + one x chunk.
Loading the two queues asymmetrically matters: they share the ramping
HBM pipe, and an overloaded scalar queue starves pair 0's late DR
phases (costs ~3-5us plus a HAM downclock). gpsimd DMAs are software-
executed (~7-11us completion) — never on the critical path.

Steady state: per 1024-token pair, x rides ONE big DMA per dtype;
per-block j-then-DR accumulation into 4 banks, eviction (scale via
ACTIVATE/TENSOR_SCALAR alternating scalar/vector engines) overlaps the
other block, one [128,4,1024] store per pair. Semaphore teardown
(~250 clears) and preamble are fixed costs — DMA-count reduction does
NOT shrink them (the full semaphore file is always reset).
"""

import os
from contextlib import ExitStack

import numpy as np
import ml_dtypes

import concourse.bass as bass
import concourse.mybir as mybir
import concourse.tile as tile
from concourse import bacc, bass_utils

P = 128                 # SBUF partitions / PE array dim
D_IN = 4096             # contraction dim (in features)
D_OUT = 4096            # out features
M_TOT = 8192            # tokens (4*2048)
N_CORES = 8
N_SHARD = D_OUT // N_CORES      # 512 out features per core
K_TILES = D_IN // P             # 32
NB = 14                         # fp16 k-tiles (0..NB-1)
NFP = (K_TILES - NB) // 2       # 9 fp8 DoubleRow k-tile pairs
M_BLK = 512                     # moving free dim per matmul
M_BLKS = M_TOT // M_BLK         # 16
M_PAIRS = M_BLKS // 2           # 8 (x is loaded in block pairs)
N_TILES = N_SHARD // P          # 4
PAIR_W = 2 * M_BLK              # 1024

_CACHE = {}
LAST_RESULTS = None  # BassKernelResults of the most recent run (for test harness)


def _install_ntff_hook():
    """Register the ctypes NTFF profiling hook under antenv.axon_hooks so
    run_bass_kernel_spmd(trace=True) can capture device profiles under axon.
    No-op if already present or the .so lacks the symbols."""
    import contextlib
    import ctypes
    import sys
    import types

    try:
        from antenv.axon_hooks import get_axon_ntff_profile_hook  # noqa: F401

        return True
    except ImportError:
        pass

    so_path = "/opt/axon/libaxon_pjrt.so"
    if not os.path.exists(so_path):
        return False
    lib = ctypes.CDLL(so_path)
    if not hasattr(lib, "axon_start_nrt_profile"):
        return False
    lib.axon_start_nrt_profile.argtypes = [
        ctypes.POINTER(ctypes.c_int64),
        ctypes.c_size_t,
    ]
    lib.axon_start_nrt_profile.restype = ctypes.c_int64
    lib.axon_stop_nrt_profile.argtypes = [ctypes.c_char_p]
    lib.axon_stop_nrt_profile.restype = ctypes.c_int64

    @contextlib.contextmanager
    def _hook(output_dir, device_ids):
        import jax

        jax.devices()
        if device_ids:
            ids = (ctypes.c_int64 * len(device_ids))(*device_ids)
            rc = lib.axon_start_nrt_profile(ids, len(device_ids))
        else:
            rc = lib.axon_start_nrt_profile(None, 0)
        if rc != 0:
            raise RuntimeError(f"axon_start_nrt_profile rc={rc}")
        try:
            yield
        finally:
            n = lib.axon_stop_nrt_profile(str(output_dir).encode())
            print(f"ntff profile: {n} file(s) written to {output_dir}")

    mod = types.ModuleType("antenv.axon_hooks")
    _state = {"hook": _hook}
    mod.set_axon_ntff_profile_hook = lambda h: _state.__setitem__("hook", h)
    mod.get_axon_ntff_profile_hook = lambda: _state["hook"]
    sys.modules["antenv.axon_hooks"] = mod
    import antenv

    antenv.axon_hooks = mod

    # artifact upload reaches for a cloud bucket that isn't available here
    bass_utils.upload_artifacts = lambda tmpdir: f"local:{tmpdir}"
    return True


def _build_nc():
    nc = bacc.Bacc(
        "TRN2", target_bir_lowering=False, debug=False, num_devices=N_CORES,
        enable_partition_id=False,
    )
    xH = nc.dram_tensor(
        "xH", [M_PAIRS, P, NB, PAIR_W], mybir.dt.float16, kind="ExternalInput"
    )
    xF = nc.dram_tensor(
        "xF", [M_PAIRS, P, NFP, 2, PAIR_W], mybir.dt.float8e4,
        kind="ExternalInput",
    )
    sgB = nc.dram_tensor(
        "sgB", [P, NB * N_SHARD], mybir.dt.float16, kind="ExternalInput"
    )
    sgF = nc.dram_tensor(
        "sgF", [P, NFP * 2 * N_SHARD], mybir.dt.float8e4, kind="ExternalInput"
    )
    sc = nc.dram_tensor("sc", [P, N_TILES], mybir.dt.float32, kind="ExternalInput")
    outT = nc.dram_tensor(
        "outT", [P, N_TILES, M_TOT], mybir.dt.float16, kind="ExternalOutput"
    )

    with tile.TileContext(nc) as tc, ExitStack() as ctx:
        sb = ctx.enter_context(tc.tile_pool(name="sb", bufs=1))
        pp = ctx.enter_context(tc.tile_pool(name="psum", bufs=1, space="PSUM"))

        # Every tile allocated exactly once; reuse is explicit by parity.
        sgB_t = sb.tile([P, NB, N_SHARD], mybir.dt.float16)
        sgF_t = sb.tile([P, NFP, 2, N_SHARD], mybir.dt.float8e4)
        sct = sb.tile([P, N_TILES], mybir.dt.float32)
        warm = sb.tile([P, P + M_BLK], mybir.dt.float16)
        # x tiles are TRIPLE-buffered: with 2 buffers pair q's load can only
        # start after pair q-2 is fully read and completes just-in-time,
        # costing ~0.4us of semaphore latency at every other pair boundary.
        xbs = [
            sb.tile([P, NB, PAIR_W], mybir.dt.float16, name=f"xb{i}")
            for i in range(3)
        ]
        xfs = [
            sb.tile([P, NFP, 2, PAIR_W], mybir.dt.float8e4, name=f"xf{i}")
            for i in range(3)
        ]
        ops = [
            sb.tile([P, N_TILES, PAIR_W], mybir.dt.float16, name=f"op{i}")
            for i in range(2)
        ]
        # 8 PSUM banks as two 4-bank sets; block g (= 2q+b) uses set g%2.
        psums = [
            [
                pp.tile([P, M_BLK], mybir.dt.float32, name=f"ps{i}_{ni}")
                for ni in range(N_TILES)
            ]
            for i in range(2)
        ]

        # Per-queue DMA emission-order chains.
        prev_dma = {}

        def qload(queue, qname, dst, src):
            dma = queue.dma_start(dst, src)
            if prev_dma.get(qname) is not None:
                tile.add_dep_helper(
                    dma.ins, prev_dma[qname].ins, sync=False,
                    reason="DMA queue emission order",
                )
            prev_dma[qname] = dma
            return dma

        # ---- HAM warmup: three dummy matmuls with no DMA dependencies give
        # the PE clock-gate monitor activity credit while the first x/sign
        # chunks land; sized to finish as the operands become consumable.
        # ~3.4us of back-to-back cold-clock warmups — exactly the HAM ramp
        # window — so the real stream starts at full clock; an idle gap
        # between warmups and the first data-ready matmul resets the ramp
        # credit. The tail warmups are 256-wide so the first real matmul
        # slots in behind the in-order queue with finer granularity.
        nc.vector.memset(warm[:], 0.0)
        for wi, w in enumerate((M_BLK,) * 5 + (M_BLK // 2,) * 5):
            nc.tensor.matmul(
                psums[0][wi % 3][:, 0:w], warm[:, 0:P], warm[:, P : P + w],
                start=True, stop=True,
            )

        xb0, xf0 = xbs[0], xfs[0]
        # The DMA/HBM path itself ramps (~45GB/s cold to ~165+GB/s after
        # ~15us), so pair 0 is consumed j-outer across BOTH blocks (half the
        # supply rate of the per-block order) and its x arrives as full-j
        # 256KB chunks (2KB contiguous per partition) — except j0, split in
        # halves so the first matmul fires ~2.7us earlier. (gpsimd DMAs are
        # software-executed with multi-us latency — never on the critical
        # path.)
        # Early supply is limited by the cold-HBM per-queue DMA rate
        # (~45GB/s ramping to ~165+), and only sync + scalar have hardware
        # DMA queues; weave a third of the x chunks into the scalar queue
        # between the (half-sized) sign chunks so both queues carry ~equal
        # bytes per j-phase, everything in consumption order.
        qload(nc.sync, "y", xb0[:, 0, 0:M_BLK], xH[0, :, 0, 0:M_BLK])
        qload(nc.sync, "y", xb0[:, 0, M_BLK:PAIR_W], xH[0, :, 0, M_BLK:PAIR_W])
        for j in (1, 2, 3, 5, 6, 7):
            qload(nc.sync, "y", xb0[:, j, :], xH[0, :, j, :])
        for j0 in range(8, NB, 2):
            j1 = min(j0 + 2, NB)
            qload(nc.sync, "y", xb0[:, j0:j1, :], xH[0, :, j0:j1, :])
        for c in range(3):
            qload(nc.sync, "y", xf0[:, c, :, :], xF[0, :, c, :, :])
        for c0 in range(3, NFP, 3):
            c1 = min(c0 + 3, NFP)
            qload(nc.sync, "y", xf0[:, c0:c1, :, :], xF[0, :, c0:c1, :, :])
        # Scalar: sct first — it's 2KB but the first eviction (~52us) needs
        # it, and at the chain's tail it completes WAY too late (~55us) —
        # then signs in consumption order. Don't put x chunks here: the two
        # queues share the ramping HBM pipe and a loaded scalar queue
        # starves q0's late DR phases.
        qload(nc.scalar, "s", sct[:], sc[:, :])
        qload(nc.scalar, "s", sgB_t[:, 0, :], sgB[:, 0:N_SHARD])
        qload(nc.scalar, "s", sgB_t[:, 1, :], sgB[:, N_SHARD : 2 * N_SHARD])
        qload(nc.scalar, "s", sgB_t[:, 2:4, :], sgB[:, 2 * N_SHARD : 4 * N_SHARD])
        # one x chunk rides scalar (after the signs it must not delay) to
        # relieve the cold sync queue mid-startup
        qload(nc.scalar, "s", xb0[:, 4, :], xH[0, :, 4, :])
        qload(nc.scalar, "s", sgB_t[:, 4:7, :], sgB[:, 4 * N_SHARD : 7 * N_SHARD])
        qload(nc.scalar, "s", sgB_t[:, 7:11, :], sgB[:, 7 * N_SHARD : 11 * N_SHARD])
        qload(nc.scalar, "s", sgB_t[:, 11:NB, :], sgB[:, 11 * N_SHARD : NB * N_SHARD])
        qload(nc.scalar, "s", sgF_t[:, 0:3, :, :], sgF[:, 0 : 6 * N_SHARD])
        qload(nc.scalar, "s", sgF_t[:, 3:6, :, :], sgF[:, 6 * N_SHARD : 12 * N_SHARD])
        qload(nc.scalar, "s", sgF_t[:, 6:NFP, :, :], sgF[:, 12 * N_SHARD :])

        def issue_x_pair(q):
            # Both x DMAs stay on the sync queue: routing xf via scalar
            # parks it behind store_pair(q) (which blocks until pair q's
            # evictions), making xf borderline-late every pair — stalls
            # plus a HAM downclock cascade (~+60us!).
            xb, xf = xbs[q % 3], xfs[q % 3]
            qload(nc.sync, "y", xb[:, :, :], xH[q, :, :, :])
            qload(nc.sync, "y", xf[:, :, :, :], xF[q, :, :, :, :])
            return xb, xf

        def mm_b(pss, xb, b, ni, j, start=None, stop=False):
            nc.tensor.matmul(
                pss[ni][:],
                sgB_t[:, j, ni * P : (ni + 1) * P],
                xb[:, j, b * M_BLK : (b + 1) * M_BLK],
                start=(j == 0) if start is None else start,
                stop=stop,
            )

        def mm_f(pss, xf, b, ni, jj, start=False, stop=None):
            nc.tensor.matmul(
                pss[ni][:],
                sgF_t[:, jj, :, ni * P : (ni + 1) * P],
                xf[:, jj, :, b * M_BLK : (b + 1) * M_BLK],
                start=start,
                stop=(jj == NFP - 1) if stop is None else stop,
                perf_mode=mybir.MatmulPerfMode.DoubleRow,
            )

        def do_block(pss, xb, xf, b, dr_first, ni_list=None):
            # Adjacent blocks run their fp16/DR sections in opposite order
            # so block seams meet same-mode (measured neutral — the per-pair
            # hiccups are DMA-burst-locked, not mode-switch — but harmless);
            # start/stop accumulation flags ride whichever section runs
            # first/last.
            nis = range(N_TILES) if ni_list is None else ni_list
            if dr_first:
                for jj in range(NFP):
                    for ni in nis:
                        mm_f(pss, xf, b, ni, jj, start=(jj == 0), stop=False)
                for j in range(NB):
                    for ni in nis:
                        mm_b(pss, xb, b, ni, j, start=False, stop=(j == NB - 1))
            else:
                for j in range(NB):
                    for ni in nis:
                        mm_b(pss, xb, b, ni, j)
                for jj in range(NFP):
                    for ni in nis:
                        mm_f(pss, xf, b, ni, jj)

        def evict_block(pss, op, b):
            # Evictions alternate between the scalar and vector engines so
            # the per-block eviction chain (and the kernel tail) is half as
            # long.
            for ni in range(N_TILES):
                dst = op[:, ni, b * M_BLK : (b + 1) * M_BLK]
                if ni % 2 == 0:
                    nc.scalar.mul(dst, pss[ni][:], sct[:, ni : ni + 1])
                else:
                    nc.vector.tensor_scalar_mul(dst, pss[ni][:], sct[:, ni : ni + 1])

        def store_pair(q, op):
            qload(
                nc.scalar, "s",
                outT[:, :, q * PAIR_W : (q + 1) * PAIR_W], op[:, :, :],
            )

        # ---- Main loop
        for q in range(M_PAIRS):
            xb, xf = (xb0, xf0) if q == 0 else issue_x_pair(q)
            op = ops[q % 2]
            if q == 0:
                # j-outer across BOTH blocks (8 PSUM banks) so the PE keeps
                # pace with the ramping HBM-limited startup stream.
                for j in range(NB):
                    for b in range(2):
                        for ni in range(N_TILES):
                            mm_b(psums[b], xb, b, ni, j)
                for jj in range(NFP):
                    # natural b-order: psums[0]'s stop lands 8 instructions
                    # before q0's end, so its eviction overlaps the tail and
                    # pair 1's first matmuls start on freed banks
                    for b in range(2):
                        for ni in range(N_TILES):
                            mm_f(psums[b], xf, b, ni, jj)
                for b in range(2):
                    evict_block(psums[b], op, b)
                store_pair(q, op)
            elif q < M_PAIRS - 1:
                for b in range(2):
                    pss = psums[b]
                    do_block(pss, xb, xf, b, dr_first=(b == 1))
                    evict_block(pss, op, b)
                store_pair(q, op)
            else:
                # Final pair: block 0 stores as one chunk as soon as its
                # eviction completes; block 1 runs ni-outer so each n-tile's
                # stop matmul lands early and its eviction + store overlap
                # the remaining matmuls; the very last n-tile's eviction and
                # store are split across both engines / two DMA queues.
                pss = psums[0]
                do_block(pss, xb, xf, 0, dr_first=False)
                evict_block(pss, op, 0)
                qload(
                    nc.scalar, "s",
                    outT[:, :, q * PAIR_W : q * PAIR_W + M_BLK],
                    op[:, :, 0:M_BLK],
                )
                pss = psums[1]
                tailq = [(nc.scalar, "s"), (nc.sync, "y"), (nc.scalar, "s")]
                for ni in range(N_TILES):
                    # per-ni alternation keeps every ni seam same-mode
                    do_block(pss, xb, xf, 1, dr_first=(ni % 2 == 0), ni_list=[ni])
                    c0 = q * PAIR_W + M_BLK
                    if ni < N_TILES - 1:
                        dst = op[:, ni, M_BLK:PAIR_W]
                        if ni % 2 == 0:
                            nc.scalar.mul(dst, pss[ni][:], sct[:, ni : ni + 1])
                        else:
                            nc.vector.tensor_scalar_mul(
                                dst, pss[ni][:], sct[:, ni : ni + 1]
                            )
                        eng, en = tailq[ni]
                        qload(
                            eng, en,
                            outT[:, ni, c0 : c0 + M_BLK],
                            op[:, ni, M_BLK:PAIR_W],
                        )
                    else:
                        # split the last eviction + store across engines
                        H = M_BLK // 2
                        nc.scalar.mul(
                            op[:, ni, M_BLK : M_BLK + H],
                            pss[ni][:, 0:H], sct[:, ni : ni + 1],
                        )
                        nc.vector.tensor_scalar_mul(
                            op[:, ni, M_BLK + H : PAIR_W],
                            pss[ni][:, H:M_BLK], sct[:, ni : ni + 1],
                        )
                        qload(
                            nc.scalar, "s",
                            outT[:, ni, c0 : c0 + H],
                            op[:, ni, M_BLK : M_BLK + H],
                        )
                        qload(
                            nc.sync, "y",
                            outT[:, ni, c0 + H : c0 + M_BLK],
                            op[:, ni, M_BLK + H : PAIR_W],
                        )

    nc.compile()
    return nc


def kernel(x, weight):
    global LAST_RESULTS
    nc = _CACHE.get("nc")
    if nc is None:
        nc = _CACHE["nc"] = _build_nc()

    x = np.asarray(x)
    weight = np.asarray(weight)
    orig_shape = x.shape

    KB = NB * P  # contraction cols in fp16

    # Host-side layout: x.T pre-tiled, partition-major so each pair is one
    # contiguous [128, *] DMA; fp16 for k-tiles 0..NB-1, e4m3 for the
    # DoubleRow k-tile pairs.
    xT = x.reshape(M_TOT, D_IN).T  # [D_IN, M_TOT] view
    # [q, p, j*1024 + c] = xT[j*128+p, q*1024+c]
    xH = np.ascontiguousarray(
        xT[:KB].reshape(NB, P, M_PAIRS, PAIR_W)
        .transpose(2, 1, 0, 3)
        .reshape(M_PAIRS, P, NB * PAIR_W)
        .astype(np.float16)
    )
    # [q, p, jj*2048 + t*1024 + c] = xT[KB + (2jj+t)*128 + p, q*1024+c]
    xF = np.ascontiguousarray(
        xT[KB:].reshape(NFP, 2, P, M_PAIRS, PAIR_W)
        .transpose(3, 2, 0, 1, 4)
        .reshape(M_PAIRS, P, NFP * 2 * PAIR_W)
        .astype(ml_dtypes.float8_e4m3fn)
    )

    SgT = np.sign(weight.T)  # [D_IN, D_OUT] f32, sign exact
    s_full = np.abs(weight.astype(np.float64)).mean(axis=1).astype(np.float32)

    in_maps = []
    for c in range(N_CORES):
        n0 = c * N_SHARD
        shard = SgT[:, n0 : n0 + N_SHARD]  # [D_IN, 512]
        # sgB[p, j*512+n] = sign(wT[j*128+p, n0+n])
        sgB = np.ascontiguousarray(
            shard[:KB].reshape(NB, P, N_SHARD)
            .transpose(1, 0, 2)
            .reshape(P, NB * N_SHARD)
            .astype(np.float16)
        )
        # sgF[p, jj*1024 + t*512 + n] = sign(wT[(NB+2jj+t)*128+p, n0+n])
        sgF = np.ascontiguousarray(
            shard[KB:].reshape(NFP, 2, P, N_SHARD)
            .transpose(2, 0, 1, 3)
            .reshape(P, NFP * 2 * N_SHARD)
            .astype(ml_dtypes.float8_e4m3fn)
        )
        scl = np.ascontiguousarray(
            s_full[n0 : n0 + N_SHARD].reshape(N_TILES, P).T
        )  # [128, 4] f32
        in_maps.append({"xH": xH, "xF": xF, "sgB": sgB, "sgF": sgF, "sc": scl})

    trace = bool(int(os.environ.get("BITLIN_TRACE", "0")))
    if trace:
        trace = _install_ntff_hook()
        base = os.environ.get("BITLIN_TRACE_DIR") or None
        if base:
            import tempfile

            os.makedirs(base, exist_ok=True)
            tmpdir = tempfile.mkdtemp(dir=base)
        else:
            tmpdir = None
    else:
        tmpdir = None
    res = bass_utils.run_bass_kernel_spmd(
        nc, in_maps, core_ids=list(range(N_CORES)), trace=trace, tmpdir=tmpdir
    )
    LAST_RESULTS = res

    # outT[c] is [128, 4, 8192] fp16 with feature index = ni*128 + p.
    outT_full = np.concatenate(
        [
            np.asarray(res.results[c]["outT"]).transpose(1, 0, 2).reshape(
                N_SHARD, M_TOT
            )
            for c in range(N_CORES)
        ],
        axis=0,
    )  # [D_OUT, M_TOT] fp16
    out = (
        np.ascontiguousarray(outT_full.T).astype(np.float32).reshape(orig_shape)
    )
    return out
